# revision 1
# baseline (speedup 1.0000x reference)
"""Trainium2 Bass kernel: row-wise sort-by-(x*rho), clamp vs -c, unsort.

Math: out[b, j] = max(x[b, j], -c[rank[b, j]]) where rank[b, j] is the
(stable) rank of key x[b,j]*rho[b,j] within row b. Implemented per 128-row
tile as: keys = x*rho; bitonic argsort carrying a u16 index payload
(all-ascending "flip" network, 91 stages); a few odd-even passes to restore
stable tie order; then -c (split into u16 hi/lo halves) is scattered to the
original positions with GPSIMD local_scatter (rank i's value -c[i] lands at
column idx_sorted[i]); recombine and take max with x.

Sharding: data-parallel over the batch dim, 4096 rows -> 8 cores x 512 rows.
"""
import sys

sys.path.insert(0, "/opt/trn_rl_repo")

import numpy as np
import concourse.bass as bass
import concourse.tile as tile
from concourse import bacc, mybir
from concourse.bass import AP
from concourse.bass_utils import run_bass_kernel_spmd

F32 = mybir.dt.float32
U16 = mybir.dt.uint16
I16 = mybir.dt.int16
U8 = mybir.dt.uint8
ALU = mybir.AluOpType

B = 4096
P = 8192
N_CORES = 8
ROWS_PER_CORE = B // N_CORES
CHUNK = 1024
N_TIEFIX = 4


def build_program(rows=ROWS_PER_CORE, p=P, n_tiefix=N_TIEFIX, chunk=CHUNK):
    assert rows % 128 == 0 and (p & (p - 1)) == 0
    ntiles = rows // 128
    nchunks = (p + chunk - 1) // chunk
    assert chunk * 32 < 2**16 and chunk % 2 == 0

    nc = bacc.Bacc("TRN2", target_bir_lowering=False, debug=False)
    x_d = nc.dram_tensor("x", [rows, p], F32, kind="ExternalInput")
    rho_d = nc.dram_tensor("rho", [rows, p], F32, kind="ExternalInput")
    c_d = nc.dram_tensor("c", [p], F32, kind="ExternalInput")
    out_d = nc.dram_tensor("out", [rows, p], F32, kind="ExternalOutput")

    with tile.TileContext(nc) as tc:
        with (
            tc.tile_pool(name="persist", bufs=1) as persist,
            tc.tile_pool(name="big", bufs=2) as big,
            tc.tile_pool(name="idxp", bufs=1) as idx_pool,
            tc.tile_pool(name="scratch", bufs=1) as scratch,
            tc.tile_pool(name="mask", bufs=1) as mask_pool,
        ):
            negc = big.tile([128, p], F32, tag="k")
            nc.sync.dma_start(negc[0:1, :], c_d.ap().unsqueeze(0))
            nc.vector.tensor_scalar_mul(negc[0:1, :], negc[0:1, :], -1.0)
            nc.gpsimd.partition_broadcast(negc[:], negc[0:1, :])
            negc_lo = persist.tile([128, p], U16, tag="negc_lo")
            negc_hi = persist.tile([128, p], U16, tag="negc_hi")
            negc_pairs = negc[:].bitcast(U16).rearrange(
                "q (n two) -> q n two", two=2)
            nc.vector.tensor_copy(negc_lo[:], negc_pairs[:, :, 0:1].squeeze(2))
            nc.vector.tensor_copy(negc_hi[:], negc_pairs[:, :, 1:2].squeeze(2))

            for t in range(ntiles):
                rs = slice(t * 128, (t + 1) * 128)
                xt = big.tile([128, p], F32, tag="k")
                rhot = scratch.tile([128, p], F32, tag="s1")
                nc.sync.dma_start(xt[:], x_d.ap()[rs, :])
                nc.sync.dma_start(rhot[:], rho_d.ap()[rs, :])

                kcur = big.tile([128, p], F32, tag="k")
                nc.vector.tensor_tensor(kcur[:], xt[:], rhot[:], ALU.mult)

                idx = idx_pool.tile([128, p], U16, tag="idx")
                nc.gpsimd.iota(idx[:], pattern=[[1, p]], channel_multiplier=0)

                def pair_views(tile_ap, kind, k=None, j=None):
                    h = tile_ap.tensor
                    part = list(tile_ap.ap[0])
                    if kind == "flip":
                        a = AP(h, tile_ap.offset, [part, [k, p // k], [1, k // 2]])
                        b = AP(h, tile_ap.offset + (k - 1),
                               [part, [k, p // k], [-1, k // 2]])
                    else:
                        a = AP(h, tile_ap.offset,
                               [part, [2 * j, p // (2 * j)], [1, j]])
                        b = AP(h, tile_ap.offset + j,
                               [part, [2 * j, p // (2 * j)], [1, j]])
                    return a, b

                def cmp_exchange(kind, k=None, j=None):
                    nonlocal kcur
                    kA, kB = pair_views(kcur[:], kind, k, j)
                    knew = big.tile([128, p], F32, tag="k")
                    nkA, nkB = pair_views(knew[:], kind, k, j)
                    iA, iB = pair_views(idx[:], kind, k, j)
                    m = mask_pool.tile([128, p], U8, tag="m")
                    mv = pair_views(m[:], kind, k, j)[0]
                    tmp = mask_pool.tile([128, p], U16, tag="tmp")
                    tv = pair_views(tmp[:], kind, k, j)[0]
                    nc.vector.tensor_tensor(mv, kA, kB, ALU.is_gt)
                    nc.vector.tensor_tensor(nkA, kA, kB, ALU.min)
                    nc.vector.tensor_tensor(nkB, kA, kB, ALU.max)
                    nc.scalar.copy(tv, iA)
                    nc.vector.copy_predicated(iA, mv, iB)
                    nc.vector.copy_predicated(iB, mv, tv)
                    kcur = knew

                k = 2
                while k <= p:
                    cmp_exchange("flip", k=k)
                    j = k // 4
                    while j >= 1:
                        cmp_exchange("uniform", j=j)
                        j //= 2
                    k *= 2

                def tiefix(offset):
                    npair = (p - offset) // 2

                    def sview(tl, off):
                        return AP(tl[:].tensor, tl[:].offset + off,
                                  [list(tl[:].ap[0]), [2, npair]])

                    kA = sview(kcur, offset)
                    kB = sview(kcur, offset + 1)
                    iA = sview(idx, offset)
                    iB = sview(idx, offset + 1)
                    meq_t = mask_pool.tile([128, p], U8, tag="m")
                    mgt_t = mask_pool.tile([128, p], U8, tag="mgt")
                    tmp2_t = mask_pool.tile([128, p], U16, tag="tmp")
                    meq, mgt, tmp2 = (sview(meq_t, 0), sview(mgt_t, 0),
                                      sview(tmp2_t, 0))
                    nc.vector.tensor_tensor(meq, kA, kB, ALU.is_ge)
                    nc.vector.tensor_tensor(mgt, iA, iB, ALU.is_gt)
                    nc.vector.tensor_tensor(meq, meq, mgt, ALU.mult)
                    nc.scalar.copy(tmp2, iA)
                    nc.vector.copy_predicated(iA, meq, iB)
                    nc.vector.copy_predicated(iB, meq, tmp2)

                for q in range(n_tiefix):
                    tiefix(q % 2)

                vlo = big.tile([128, p], U16, tag="k")
                vhi = big.tile([128, p], U16, tag="k")
                idx_i16 = idx[:].bitcast(I16)
                for ci in range(nchunks):
                    q1 = mask_pool.tile([128, p], I16, tag="tmp")
                    q2 = mask_pool.tile([128, p], I16, tag="q2")
                    nc.vector.tensor_scalar(q1[:], idx_i16,
                                            float(chunk * (ci + 1)),
                                            float(-2 * p), ALU.is_ge, ALU.mult)
                    nc.vector.scalar_tensor_tensor(
                        q2[:], idx_i16, float(-chunk * ci), q1[:],
                        ALU.add, ALU.add)
                    nc.gpsimd.local_scatter(
                        vlo[:, ci * chunk:(ci + 1) * chunk], negc_lo[:], q2[:],
                        channels=128, num_elems=chunk, num_idxs=p)
                    nc.gpsimd.local_scatter(
                        vhi[:, ci * chunk:(ci + 1) * chunk], negc_hi[:], q2[:],
                        channels=128, num_elems=chunk, num_idxs=p)

                v = scratch.tile([128, p], F32, tag="s1")
                v_pairs = v[:].bitcast(U16).rearrange(
                    "q (n two) -> q n two", two=2)
                nc.vector.tensor_copy(v_pairs[:, :, 0:1].squeeze(2), vlo[:])
                nc.vector.tensor_copy(v_pairs[:, :, 1:2].squeeze(2), vhi[:])
                xt2 = big.tile([128, p], F32, tag="k")
                nc.sync.dma_start(xt2[:], x_d.ap()[rs, :])
                nc.vector.tensor_tensor(v[:], v[:], xt2[:], ALU.max)
                nc.sync.dma_start(out_d.ap()[rs, :], v[:])

    nc.compile()
    return nc


_CACHED_NC = None


def _get_nc():
    global _CACHED_NC
    if _CACHED_NC is None:
        _CACHED_NC = build_program()
    return _CACHED_NC


def kernel(x, rho, c, _trace=False, _trace_kwargs=None):
    x = np.ascontiguousarray(np.asarray(x, dtype=np.float32))
    rho = np.ascontiguousarray(np.asarray(rho, dtype=np.float32))
    c = np.ascontiguousarray(np.asarray(c, dtype=np.float32))
    assert x.shape == (B, P) and rho.shape == (B, P) and c.shape == (P,)

    nc = _get_nc()
    in_maps = []
    for i in range(N_CORES):
        rs = slice(i * ROWS_PER_CORE, (i + 1) * ROWS_PER_CORE)
        in_maps.append({"x": x[rs], "rho": rho[rs], "c": c})
    res = run_bass_kernel_spmd(nc, in_maps, list(range(N_CORES)),
                               trace=_trace, **(_trace_kwargs or {}))
    out = np.concatenate([res.results[i]["out"] for i in range(N_CORES)], axis=0)
    if _trace:
        return out, res
    return out

